# revision 35
# baseline (speedup 1.0000x reference)
"""Trainium2 Bass kernel for nn_IntraClassLoss (segment_reduce).

Math: inputs [B,C,H,W] logits, targets [B,H,W] int labels, C=4.
probs = softmax(inputs, axis=1); for classes c in 1..C-1:
  cnt_c = #pixels with target==c
  S1_c  = sum over those pixels of p_c
  S2_c  = sum over those pixels of p_c^2
  mean_c = S1_c/(cnt_c+eps); var_c = (S2_c - 2*mean_c*S1_c + cnt_c*mean_c^2)/(cnt_c+eps)
  loss = sum_{c: cnt_c>0} var_c / (C-1)

Sharding: data-parallel over batch, 2 batches per core on 8 cores. Each core
reduces its shard to per-class S1/S2 partials which are DMA'd out and
finished on the host (cnt_c from a host-side bincount; no collectives).

Engine assignment (per [128,n] chunk; DMA of the 40MiB shard is the
roofline at ~117us, every engine is kept under it):
  DVE : tb=bf16(t) cast, masks m_c=(tb==c) (tensor_scalar 4x), recip(den),
        rbf=bf16(r), products a_c=m_c*e_c and pc_c=a_c*rbf (tensor_tensor 2x)
  ACT : exp only -- one fused op over all 4 class slices of x
  Pool: a_c for n_pool of the 3 classes (idle engine, off critical path)
  PE  : den = sum_c e_c (identity-matmul PSUM accumulation);
        S1_c via pc-stationary x ones matmuls into a [128,1] PSUM;
        S2_c via pc_slice^T @ pc_slice matmuls into a [128,128] PSUM whose
        accumulated DIAGONAL holds per-column sums of pc^2 (extracted once
        at the end with an identity mask + reduce)

The per-chunk work is emitted software-pipelined -- chunk j+1's DMA/exp/den
("head") is issued before chunk j's recip/product chain ("tail") -- so each
engine's in-order stream never makes next-chunk work wait on the previous
chunk's tail. On the final chunk, exp runs per-class interleaved with den
matmuls to shorten the post-last-DMA critical chain.
"""

import numpy as np

import concourse.bass as bass
import concourse.bacc as bacc
import concourse.tile as tile
from concourse import mybir
from concourse.bass_utils import run_bass_kernel_spmd

F32 = mybir.dt.float32
BF16 = mybir.dt.bfloat16
I32 = mybir.dt.int32
AF = mybir.ActivationFunctionType
ALU = mybir.AluOpType

B, C, H, W = 16, 4, 1024, 1024
N_CORES = 8
B_LOC = B // N_CORES
P = 128
EPS = 1e-6

CHUNKS = (512,) * 16  # per batch plane; must sum to H*W/P = 8192
N_POOL = 2  # how many of the 3 per-class a_c=m_c*e_c products run on Pool


def build_program(b_loc=B_LOC, h=H, w=W, chunks=CHUNKS, n_pool=N_POOL,
                  io_bufs=6, e_bufs=6, work_bufs=5, den_bufs=1,
                  pool_cast=False, tail_no_pool=0, n_pool_z=0):
    """Build the per-core SPMD program. Returns (nc, n_chunks_total)."""
    plane = h * w
    free = plane // P
    assert sum(chunks) == free
    all_chunks = [(b, n) for b in range(b_loc) for n in chunks]
    n_chunks = len(all_chunks)
    nmax = max(chunks)
    n_red_total = sum(n // P for _, n in all_chunks)  # PE-reduce matmuls/class

    nc = bacc.Bacc("TRN2", target_bir_lowering=False, debug=False)

    inputs_d = nc.dram_tensor("inputs", [b_loc, C, h, w], F32, kind="ExternalInput")
    targets_d = nc.dram_tensor("targets", [b_loc, h, w], I32, kind="ExternalInput")
    ident_d = nc.dram_tensor("ident", [P, P], BF16, kind="ExternalInput")
    ones_d = nc.dram_tensor("ones", [P, 1], BF16, kind="ExternalInput")
    s1_d = nc.dram_tensor("outs1", [P, 8], F32, kind="ExternalOutput")

    with tile.TileContext(nc) as tc:
        with (
            tc.tile_pool(name="const", bufs=1) as constp,
            tc.tile_pool(name="io", bufs=io_bufs) as iop,
            tc.tile_pool(name="ework", bufs=e_bufs) as ep,
            tc.tile_pool(name="work", bufs=work_bufs) as workp,
            tc.tile_pool(name="stats", bufs=1) as statp,
            tc.tile_pool(name="psum", bufs=den_bufs, space="PSUM") as psump,
            tc.tile_pool(name="psum1", bufs=1, space="PSUM") as psump1,
        ):
            ident = constp.tile([P, P], BF16)
            nc.sync.dma_start(ident[:], ident_d.ap())
            ones = constp.tile([P, 1], BF16)
            nc.sync.dma_start(ones[:], ones_d.ap())

            s1_p = [
                psump1.tile([P, 1], F32, tag=f"s1p{ci}", name=f"s1p{ci}")
                for ci in range(3)
            ]
            s2_p = [
                psump1.tile([P, P], F32, tag=f"s2p{ci}", name=f"s2p{ci}")
                for ci in range(3)
            ]
            red_done = [0, 0, 0]

            # Output staging, class-major: cols 2ci = S1_c, 2ci+1 = S2_c.
            s1sb = statp.tile([P, 8], F32, tag="s1sb")
            nc.vector.memset(s1sb[:], 0.0)

            def assemble(ci):
                diag = statp.tile([P, P], F32, tag=f"diag{ci}")
                nc.vector.tensor_copy(s1sb[:, 2 * ci : 2 * ci + 1], s1_p[ci][:])
                # diag extract + reduce in one op: (s2_p * 1) * ident, accum
                nc.vector.scalar_tensor_tensor(
                    out=diag[:], in0=s2_p[ci][:], scalar=1.0, in1=ident[:],
                    op0=ALU.mult, op1=ALU.mult,
                    accum_out=s1sb[:, 2 * ci + 1 : 2 * ci + 2],
                )


            state = {}

            def head(j):
                b, n = all_chunks[j]
                off = sum(nn for _, nn in all_chunks[:j]) % free
                sl = slice(off, off + n)

                t_tile = iop.tile([P, nmax], I32, tag="t")
                tgt_ap = targets_d.ap()[b].rearrange("(p a) w -> p (a w)", p=P)
                nc.sync.dma_start(t_tile[:, :n], tgt_ap[:, sl])
                tb = workp.tile([P, nmax], BF16, tag="tb")
                cast_eng = nc.gpsimd if pool_cast else nc.vector
                cast_eng.tensor_copy(tb[:, :n], t_tile[:, :n])
                ms = []
                for ci, c in enumerate((1, 2, 3)):
                    m = workp.tile([P, nmax], BF16, tag=f"m{ci}")
                    nc.vector.tensor_scalar(
                        m[:, :n], tb[:, :n], float(c), None, ALU.is_equal
                    )
                    ms.append(m)

                x4 = iop.tile([P, 4 * nmax], F32, tag="x4")
                for c in range(C):
                    x_ap = inputs_d.ap()[b, c].rearrange("(p a) w -> p (a w)", p=P)
                    nc.sync.dma_start(x4[:, c * n : (c + 1) * n], x_ap[:, sl])
                e4 = ep.tile([P, 4 * nmax], BF16, tag="e4")
                den = psump.tile([P, nmax], F32, tag="den")
                nsl = (n + 511) // 512
                split_here = j == n_chunks - 1
                if not split_here:
                    nc.scalar.activation(e4[:, : 4 * n], x4[:, : 4 * n], AF.Exp)
                for c in range(C):
                    if split_here:
                        nc.scalar.activation(
                            e4[:, c * n : (c + 1) * n],
                            x4[:, c * n : (c + 1) * n],
                            AF.Exp,
                        )
                    for hh in range(nsl):
                        s2 = slice(hh * 512, min((hh + 1) * 512, n))
                        nc.tensor.matmul(
                            den[:, s2],
                            ident[:],
                            e4[:, c * n + s2.start : c * n + s2.stop],
                            start=(c == 0),
                            stop=(c == C - 1),
                            skip_group_check=True,
                        )
                state[j] = (n, tb, ms, e4, den)

            def tail(j):
                n, tb, ms, e4, den = state.pop(j)

                # Pool classes first so Pool can run while DVE does recip.
                np_here = n_pool if j < n_chunks - tail_no_pool else 0
                porder = [2, 1, 0][:np_here]
                a_tiles = {}
                for ci in porder:
                    c = ci + 1
                    a = workp.tile([P, nmax], BF16, tag=f"apool{ci}")
                    nc.gpsimd.tensor_tensor(
                        a[:, :n], ms[ci][:, :n],
                        e4[:, c * n : (c + 1) * n], op=ALU.mult,
                    )
                    a_tiles[ci] = a

                r32 = workp.tile([P, nmax], F32, tag="r32")
                nc.vector.reciprocal_approx_fast(r32[:, :n], den[:, :n])
                rbf = workp.tile([P, nmax], BF16, tag="rbf")
                nc.vector.tensor_copy(rbf[:, :n], r32[:, :n])

                for ci, c in enumerate((1, 2, 3)):
                    if ci in a_tiles:
                        a = a_tiles[ci]
                    else:
                        a = workp.tile([P, nmax], BF16, tag="a")
                        nc.vector.tensor_tensor(
                            a[:, :n], ms[ci][:, :n],
                            e4[:, c * n : (c + 1) * n], op=ALU.mult,
                        )
                    pc = workp.tile([P, nmax], BF16, tag="pc")
                    z_eng = nc.gpsimd if (j < n_chunks - tail_no_pool and ci < n_pool_z) else nc.vector
                    z_eng.tensor_tensor(
                        pc[:, :n], a[:, :n], rbf[:, :n], op=ALU.mult
                    )
                    # S1 and S2 partials on PE: [128,1] += pc_s^T @ ones and
                    # [128,128] += pc_s^T @ pc_s (diagonal = sum of squares)
                    for s in range(n // P):
                        sl_pc = pc[:, s * P : (s + 1) * P]
                        st = red_done[ci] == 0
                        sp = red_done[ci] == n_red_total - 1
                        nc.tensor.matmul(
                            s1_p[ci][:], sl_pc, ones[:],
                            start=st, stop=sp, skip_group_check=True,
                        )
                        nc.tensor.matmul(
                            s2_p[ci][:], sl_pc, sl_pc,
                            start=st, stop=sp, skip_group_check=True,
                        )
                        red_done[ci] += 1
                    if red_done[ci] == n_red_total:
                        assemble(ci)

            head(0)
            for j in range(n_chunks):
                if j + 1 < n_chunks:
                    head(j + 1)
                tail(j)

            nc.sync.dma_start(s1_d.ap(), s1sb[:])

    nc.compile()
    return nc, n_chunks


_CACHED = {}


def _get_program():
    if "nc" not in _CACHED:
        _CACHED["nc"] = build_program()[0]
    return _CACHED["nc"], None


def finish_host(results, cnt):
    """results: per-core dicts with outs1 [P,8], class-major: col 2c = S1_c,
    col 2c+1 = S2_c for c in 0..2 (classes 1..3)."""
    s1 = np.zeros(3, dtype=np.float64)
    s2 = np.zeros(3, dtype=np.float64)
    for r in results:
        o = r["outs1"].astype(np.float64)
        s1 += o[:, 0:6:2].sum(axis=0)
        s2 += o[:, 1:6:2].sum(axis=0)
    mean = s1 / (cnt + EPS)
    var = (s2 - 2.0 * mean * s1 + cnt * mean * mean) / (cnt + EPS)
    intra = np.where(cnt > 0, var, 0.0).sum()
    return np.float32(intra / (C - 1))


def kernel(inputs: np.ndarray, targets: np.ndarray) -> np.ndarray:
    import ml_dtypes

    inputs = np.asarray(inputs, dtype=np.float32)
    targets = np.asarray(targets, dtype=np.int32)
    nc, _ = _get_program()
    ident = np.eye(P, dtype=ml_dtypes.bfloat16)
    ones = np.ones((P, 1), dtype=ml_dtypes.bfloat16)
    in_maps = [
        {
            "inputs": np.ascontiguousarray(inputs[i * B_LOC : (i + 1) * B_LOC]),
            "targets": np.ascontiguousarray(targets[i * B_LOC : (i + 1) * B_LOC]),
            "ident": ident,
            "ones": ones,
        }
        for i in range(N_CORES)
    ]
    res = run_bass_kernel_spmd(nc, in_maps, list(range(N_CORES)))
    stats = [res.results[i] for i in range(N_CORES)]
    cnt = np.bincount(targets.ravel(), minlength=C)[1:C].astype(np.float64)
    return finish_host(stats, cnt)


# revision 44
# speedup vs baseline: 1.0014x; 1.0014x over previous
"""Trainium2 Bass kernel for nn_IntraClassLoss (segment_reduce).

Math: inputs [B,C,H,W] logits, targets [B,H,W] int labels, C=4.
probs = softmax(inputs, axis=1); for classes c in 1..C-1:
  cnt_c = #pixels with target==c
  S1_c  = sum over those pixels of p_c
  S2_c  = sum over those pixels of p_c^2
  mean_c = S1_c/(cnt_c+eps); var_c = (S2_c - 2*mean_c*S1_c + cnt_c*mean_c^2)/(cnt_c+eps)
  loss = sum_{c: cnt_c>0} var_c / (C-1)

Sharding: data-parallel over batch, 2 batches per core on 8 cores. Each core
reduces its shard to per-class S1/S2 partials which are DMA'd out and
finished on the host (cnt_c from a host-side bincount; no collectives).

Engine assignment (per [128,n] chunk; DMA of the 40MiB shard is the
roofline at ~117us, every engine is kept under it):
  DVE : tb=bf16(t) cast, masks m_c=(tb==c) (tensor_scalar 4x), recip(den),
        rbf=bf16(r), products a_c=m_c*e_c and pc_c=a_c*rbf (tensor_tensor 2x)
  ACT : exp only -- one fused op over all 4 class slices of x
  Pool: a_c for n_pool of the 3 classes (idle engine, off critical path)
  PE  : den = sum_c e_c (identity-matmul PSUM accumulation);
        S1_c via pc-stationary x ones matmuls into a [128,1] PSUM;
        S2_c via pc_slice^T @ pc_slice matmuls into a [128,128] PSUM whose
        accumulated DIAGONAL holds per-column sums of pc^2 (extracted once
        at the end with an identity mask + reduce)

The per-chunk work is emitted software-pipelined -- chunk j+1's DMA/exp/den
("head") is issued before chunk j's recip/product chain ("tail") -- so each
engine's in-order stream never makes next-chunk work wait on the previous
chunk's tail. On the final chunk, exp runs per-class interleaved with den
matmuls to shorten the post-last-DMA critical chain.
"""

import numpy as np

import concourse.bass as bass
import concourse.bacc as bacc
import concourse.tile as tile
from concourse import mybir
from concourse.bass_utils import run_bass_kernel_spmd

F32 = mybir.dt.float32
BF16 = mybir.dt.bfloat16
I32 = mybir.dt.int32
AF = mybir.ActivationFunctionType
ALU = mybir.AluOpType

B, C, H, W = 16, 4, 1024, 1024
N_CORES = 8
B_LOC = B // N_CORES
P = 128
EPS = 1e-6

CHUNKS = (512,) * 16  # per batch plane; must sum to H*W/P = 8192
N_POOL = 2  # how many of the 3 per-class a_c=m_c*e_c products run on Pool


def build_program(b_loc=B_LOC, h=H, w=W, chunks=CHUNKS, n_pool=N_POOL,
                  io_bufs=6, e_bufs=6, work_bufs=5, den_bufs=1,
                  pool_cast=False, tail_no_pool=0, n_pool_z=0):
    """Build the per-core SPMD program. Returns (nc, n_chunks_total)."""
    plane = h * w
    free = plane // P
    assert sum(chunks) == free
    all_chunks = [(b, n) for b in range(b_loc) for n in chunks]
    n_chunks = len(all_chunks)
    nmax = max(chunks)
    n_red_total = sum(n // P for _, n in all_chunks)  # PE-reduce matmuls/class

    nc = bacc.Bacc("TRN2", target_bir_lowering=False, debug=False)

    inputs_d = nc.dram_tensor("inputs", [b_loc, C, h, w], F32, kind="ExternalInput")
    targets_d = nc.dram_tensor("targets", [b_loc, h, w], I32, kind="ExternalInput")
    consts_d = nc.dram_tensor("consts", [P, P + 1], BF16, kind="ExternalInput")
    s1_d = nc.dram_tensor("outs1", [P, 8], F32, kind="ExternalOutput")

    with tile.TileContext(nc) as tc:
        with (
            tc.tile_pool(name="const", bufs=1) as constp,
            tc.tile_pool(name="io", bufs=io_bufs) as iop,
            tc.tile_pool(name="ework", bufs=e_bufs) as ep,
            tc.tile_pool(name="work", bufs=work_bufs) as workp,
            tc.tile_pool(name="stats", bufs=1) as statp,
            tc.tile_pool(name="psum", bufs=den_bufs, space="PSUM") as psump,
            tc.tile_pool(name="psum1", bufs=1, space="PSUM") as psump1,
        ):
            consts = constp.tile([P, P + 1], BF16)
            nc.sync.dma_start(consts[:], consts_d.ap())
            ident = consts[:, :P]
            ones = consts[:, P : P + 1]

            s1_p = [
                psump1.tile([P, 1], F32, tag=f"s1p{ci}", name=f"s1p{ci}")
                for ci in range(3)
            ]
            s2_p = [
                psump1.tile([P, P], F32, tag=f"s2p{ci}", name=f"s2p{ci}")
                for ci in range(3)
            ]
            red_done = [0, 0, 0]

            # Output staging, class-major: cols 2ci = S1_c, 2ci+1 = S2_c.
            s1sb = statp.tile([P, 8], F32, tag="s1sb")
            nc.vector.memset(s1sb[:], 0.0)

            def assemble(ci):
                diag = statp.tile([P, P], F32, tag=f"diag{ci}")
                nc.vector.tensor_copy(s1sb[:, 2 * ci : 2 * ci + 1], s1_p[ci][:])
                # diag extract + reduce in one op: (s2_p * 1) * ident, accum
                nc.vector.scalar_tensor_tensor(
                    out=diag[:], in0=s2_p[ci][:], scalar=1.0, in1=ident,
                    op0=ALU.mult, op1=ALU.mult,
                    accum_out=s1sb[:, 2 * ci + 1 : 2 * ci + 2],
                )


            state = {}

            def head(j):
                b, n = all_chunks[j]
                off = sum(nn for _, nn in all_chunks[:j]) % free
                sl = slice(off, off + n)

                t_tile = iop.tile([P, nmax], I32, tag="t")
                tgt_ap = targets_d.ap()[b].rearrange("(p a) w -> p (a w)", p=P)
                nc.sync.dma_start(t_tile[:, :n], tgt_ap[:, sl])

                x4 = iop.tile([P, 4 * nmax], F32, tag="x4")
                split_here = j >= n_chunks - 2
                corder = list(range(C))
                for c in corder:
                    x_ap = inputs_d.ap()[b, c].rearrange("(p a) w -> p (a w)", p=P)
                    nc.sync.dma_start(x4[:, c * n : (c + 1) * n], x_ap[:, sl])
                e4 = ep.tile([P, 4 * nmax], BF16, tag="e4")
                den = psump.tile([P, nmax], F32, tag="den")
                nsl = (n + 511) // 512
                if not split_here:
                    nc.scalar.activation(e4[:, : 4 * n], x4[:, : 4 * n], AF.Exp)
                for k, c in enumerate(corder):
                    if split_here:
                        nc.scalar.activation(
                            e4[:, c * n : (c + 1) * n],
                            x4[:, c * n : (c + 1) * n],
                            AF.Exp,
                        )
                    for hh in range(nsl):
                        s2 = slice(hh * 512, min((hh + 1) * 512, n))
                        nc.tensor.matmul(
                            den[:, s2],
                            ident,
                            e4[:, c * n + s2.start : c * n + s2.stop],
                            start=(k == 0),
                            stop=(k == C - 1),
                            skip_group_check=True,
                        )
                # Cast+masks emitted after exp so an ACT-hosted cast slots
                # into ACT's idle shadow instead of delaying the exp.
                tb = workp.tile([P, nmax], BF16, tag="tb")
                if pool_cast == "pool":
                    nc.gpsimd.tensor_copy(tb[:, :n], t_tile[:, :n])
                elif pool_cast == "act":
                    nc.scalar.activation(tb[:, :n], t_tile[:, :n], AF.Copy)
                else:
                    nc.vector.tensor_copy(tb[:, :n], t_tile[:, :n])
                ms = []
                for ci, c in enumerate((1, 2, 3)):
                    m = workp.tile([P, nmax], BF16, tag=f"m{ci}")
                    nc.vector.tensor_scalar(
                        m[:, :n], tb[:, :n], float(c), None, ALU.is_equal
                    )
                    ms.append(m)
                state[j] = (n, tb, ms, e4, den)

            def tail(j):
                n, tb, ms, e4, den = state.pop(j)

                # Pool classes first so Pool can run while DVE does recip.
                np_here = n_pool if j < n_chunks - tail_no_pool else 0
                porder = [2, 1, 0][:np_here]
                a_tiles = {}
                for ci in porder:
                    c = ci + 1
                    a = workp.tile([P, nmax], BF16, tag=f"apool{ci}")
                    nc.gpsimd.tensor_tensor(
                        a[:, :n], ms[ci][:, :n],
                        e4[:, c * n : (c + 1) * n], op=ALU.mult,
                    )
                    a_tiles[ci] = a

                r32 = workp.tile([P, nmax], F32, tag="r32")
                nc.vector.reciprocal_approx_fast(r32[:, :n], den[:, :n])
                rbf = workp.tile([P, nmax], BF16, tag="rbf")
                nc.vector.tensor_copy(rbf[:, :n], r32[:, :n])

                for ci, c in enumerate((1, 2, 3)):
                    if ci in a_tiles:
                        a = a_tiles[ci]
                    else:
                        a = workp.tile([P, nmax], BF16, tag="a")
                        nc.vector.tensor_tensor(
                            a[:, :n], ms[ci][:, :n],
                            e4[:, c * n : (c + 1) * n], op=ALU.mult,
                        )
                    pc = workp.tile([P, nmax], BF16, tag="pc")
                    z_eng = nc.gpsimd if (j < n_chunks - tail_no_pool and ci < n_pool_z) else nc.vector
                    z_eng.tensor_tensor(
                        pc[:, :n], a[:, :n], rbf[:, :n], op=ALU.mult
                    )
                    # S1 and S2 partials on PE: [128,1] += pc_s^T @ ones and
                    # [128,128] += pc_s^T @ pc_s (diagonal = sum of squares)
                    for s in range(n // P):
                        sl_pc = pc[:, s * P : (s + 1) * P]
                        st = red_done[ci] == 0
                        sp = red_done[ci] == n_red_total - 1
                        nc.tensor.matmul(
                            s1_p[ci][:], sl_pc, ones,
                            start=st, stop=sp, skip_group_check=True,
                        )
                        nc.tensor.matmul(
                            s2_p[ci][:], sl_pc, sl_pc,
                            start=st, stop=sp, skip_group_check=True,
                        )
                        red_done[ci] += 1
                    if red_done[ci] == n_red_total:
                        assemble(ci)

            head(0)
            for j in range(n_chunks):
                if j + 1 < n_chunks:
                    head(j + 1)
                tail(j)

            nc.sync.dma_start(s1_d.ap(), s1sb[:])

    nc.compile()
    return nc, n_chunks


_CACHED = {}


def _get_program():
    if "nc" not in _CACHED:
        _CACHED["nc"] = build_program()[0]
    return _CACHED["nc"], None


def finish_host(results, cnt):
    """results: per-core dicts with outs1 [P,8], class-major: col 2c = S1_c,
    col 2c+1 = S2_c for c in 0..2 (classes 1..3)."""
    s1 = np.zeros(3, dtype=np.float64)
    s2 = np.zeros(3, dtype=np.float64)
    for r in results:
        o = r["outs1"].astype(np.float64)
        s1 += o[:, 0:6:2].sum(axis=0)
        s2 += o[:, 1:6:2].sum(axis=0)
    mean = s1 / (cnt + EPS)
    var = (s2 - 2.0 * mean * s1 + cnt * mean * mean) / (cnt + EPS)
    intra = np.where(cnt > 0, var, 0.0).sum()
    return np.float32(intra / (C - 1))


def kernel(inputs: np.ndarray, targets: np.ndarray) -> np.ndarray:
    import ml_dtypes

    inputs = np.asarray(inputs, dtype=np.float32)
    targets = np.asarray(targets, dtype=np.int32)
    nc, _ = _get_program()
    consts = np.concatenate(
        [np.eye(P, dtype=np.float32), np.ones((P, 1), dtype=np.float32)], axis=1
    ).astype(ml_dtypes.bfloat16)
    in_maps = [
        {
            "inputs": np.ascontiguousarray(inputs[i * B_LOC : (i + 1) * B_LOC]),
            "targets": np.ascontiguousarray(targets[i * B_LOC : (i + 1) * B_LOC]),
            "consts": consts,
        }
        for i in range(N_CORES)
    ]
    res = run_bass_kernel_spmd(nc, in_maps, list(range(N_CORES)))
    stats = [res.results[i] for i in range(N_CORES)]
    cnt = np.bincount(targets.ravel(), minlength=C)[1:C].astype(np.float64)
    return finish_host(stats, cnt)


# revision 45
# speedup vs baseline: 1.0067x; 1.0053x over previous
"""Trainium2 Bass kernel for nn_IntraClassLoss (segment_reduce).

Math: inputs [B,C,H,W] logits, targets [B,H,W] int labels, C=4.
probs = softmax(inputs, axis=1); for classes c in 1..C-1:
  cnt_c = #pixels with target==c
  S1_c  = sum over those pixels of p_c
  S2_c  = sum over those pixels of p_c^2
  mean_c = S1_c/(cnt_c+eps); var_c = (S2_c - 2*mean_c*S1_c + cnt_c*mean_c^2)/(cnt_c+eps)
  loss = sum_{c: cnt_c>0} var_c / (C-1)

Sharding: data-parallel over batch, 2 batches per core on 8 cores. Each core
reduces its shard to per-class S1/S2 partials which are DMA'd out and
finished on the host (cnt_c from a host-side bincount; no collectives).

Engine assignment (per [128,n] chunk; DMA of the 40MiB shard is the
roofline at ~117us, every engine is kept under it):
  DVE : tb=bf16(t) cast, masks m_c=(tb==c) (tensor_scalar 4x), recip(den),
        rbf=bf16(r), products a_c=m_c*e_c and pc_c=a_c*rbf (tensor_tensor 2x)
  ACT : exp only -- one fused op over all 4 class slices of x
  Pool: a_c for n_pool of the 3 classes (idle engine, off critical path)
  PE  : den = sum_c e_c (identity-matmul PSUM accumulation);
        S1_c via pc-stationary x ones matmuls into a [128,1] PSUM;
        S2_c via pc_slice^T @ pc_slice matmuls into a [128,128] PSUM whose
        accumulated DIAGONAL holds per-column sums of pc^2 (extracted once
        at the end with an identity mask + reduce)

The per-chunk work is emitted software-pipelined -- chunk j+1's DMA/exp/den
("head") is issued before chunk j's recip/product chain ("tail") -- so each
engine's in-order stream never makes next-chunk work wait on the previous
chunk's tail. On the final chunk, exp runs per-class interleaved with den
matmuls to shorten the post-last-DMA critical chain.
"""

import numpy as np

import concourse.bass as bass
import concourse.bacc as bacc
import concourse.tile as tile
from concourse import mybir
from concourse.bass_utils import run_bass_kernel_spmd

F32 = mybir.dt.float32
BF16 = mybir.dt.bfloat16
I32 = mybir.dt.int32
AF = mybir.ActivationFunctionType
ALU = mybir.AluOpType

B, C, H, W = 16, 4, 1024, 1024
N_CORES = 8
B_LOC = B // N_CORES
P = 128
EPS = 1e-6

CHUNKS = (512,) * 16  # per batch plane; must sum to H*W/P = 8192
N_POOL = 2  # how many of the 3 per-class a_c=m_c*e_c products run on Pool


def build_program(b_loc=B_LOC, h=H, w=W, chunks=CHUNKS, n_pool=N_POOL,
                  io_bufs=6, e_bufs=6, work_bufs=5, den_bufs=1,
                  pool_cast=False, tail_no_pool=0, n_pool_z=0):
    """Build the per-core SPMD program. Returns (nc, n_chunks_total)."""
    plane = h * w
    free = plane // P
    assert sum(chunks) == free
    all_chunks = [(b, n) for b in range(b_loc) for n in chunks]
    n_chunks = len(all_chunks)
    nmax = max(chunks)
    n_red_total = sum(n // P for _, n in all_chunks)  # PE-reduce matmuls/class

    nc = bacc.Bacc("TRN2", target_bir_lowering=False, debug=False)

    inputs_d = nc.dram_tensor("inputs", [b_loc, C, h, w], F32, kind="ExternalInput")
    targets_d = nc.dram_tensor("targets", [b_loc, h, w], I32, kind="ExternalInput")
    consts_d = nc.dram_tensor("consts", [P, P + 1], BF16, kind="ExternalInput")
    s1_d = nc.dram_tensor("outs1", [P, 8], F32, kind="ExternalOutput")

    with tile.TileContext(nc) as tc:
        with (
            tc.tile_pool(name="const", bufs=1) as constp,
            tc.tile_pool(name="io", bufs=io_bufs) as iop,
            tc.tile_pool(name="ework", bufs=e_bufs) as ep,
            tc.tile_pool(name="work", bufs=work_bufs) as workp,
            tc.tile_pool(name="stats", bufs=1) as statp,
            tc.tile_pool(name="psum", bufs=den_bufs, space="PSUM") as psump,
            tc.tile_pool(name="psum1", bufs=1, space="PSUM") as psump1,
        ):
            consts = constp.tile([P, P + 1], BF16)
            nc.sync.dma_start(consts[:], consts_d.ap())
            ident = consts[:, :P]
            ones = consts[:, P : P + 1]

            s1_p = [
                psump1.tile([P, 1], F32, tag=f"s1p{ci}", name=f"s1p{ci}")
                for ci in range(3)
            ]
            s2_p = [
                psump1.tile([P, P], F32, tag=f"s2p{ci}", name=f"s2p{ci}")
                for ci in range(3)
            ]
            red_done = [0, 0, 0]

            # Output staging, class-major: cols 2ci = S1_c, 2ci+1 = S2_c.
            s1sb = statp.tile([P, 8], F32, tag="s1sb")
            nc.vector.memset(s1sb[:], 0.0)

            def assemble(ci):
                diag = statp.tile([P, P], F32, tag=f"diag{ci}")
                nc.vector.tensor_copy(s1sb[:, 2 * ci : 2 * ci + 1], s1_p[ci][:])
                # diag extract + reduce in one op: (s2_p * 1) * ident, accum
                nc.vector.scalar_tensor_tensor(
                    out=diag[:], in0=s2_p[ci][:], scalar=1.0, in1=ident,
                    op0=ALU.mult, op1=ALU.mult,
                    accum_out=s1sb[:, 2 * ci + 1 : 2 * ci + 2],
                )


            state = {}

            def head(j):
                b, n = all_chunks[j]
                off = sum(nn for _, nn in all_chunks[:j]) % free
                sl = slice(off, off + n)

                t_tile = iop.tile([P, nmax], I32, tag="t")
                tgt_ap = targets_d.ap()[b].rearrange("(p a) w -> p (a w)", p=P)
                nc.sync.dma_start(t_tile[:, :n], tgt_ap[:, sl])

                x4 = iop.tile([P, 4 * nmax], F32, tag="x4")
                split_here = j >= n_chunks - 2
                corder = list(range(C))
                for c in corder:
                    x_ap = inputs_d.ap()[b, c].rearrange("(p a) w -> p (a w)", p=P)
                    nc.sync.dma_start(x4[:, c * n : (c + 1) * n], x_ap[:, sl])
                e4 = ep.tile([P, 4 * nmax], BF16, tag="e4")
                den = psump.tile([P, nmax], F32, tag="den")
                nsl = (n + 511) // 512
                if not split_here:
                    nc.scalar.activation(e4[:, : 4 * n], x4[:, : 4 * n], AF.Exp)
                for k, c in enumerate(corder):
                    if split_here:
                        nc.scalar.activation(
                            e4[:, c * n : (c + 1) * n],
                            x4[:, c * n : (c + 1) * n],
                            AF.Exp,
                        )
                    for hh in range(nsl):
                        s2 = slice(hh * 512, min((hh + 1) * 512, n))
                        nc.tensor.matmul(
                            den[:, s2],
                            ident,
                            e4[:, c * n + s2.start : c * n + s2.stop],
                            start=(k == 0),
                            stop=(k == C - 1),
                            skip_group_check=True,
                        )
                # Cast+masks emitted after exp so an ACT-hosted cast slots
                # into ACT's idle shadow instead of delaying the exp.
                tb = workp.tile([P, nmax], BF16, tag="tb")
                if pool_cast == "pool":
                    nc.gpsimd.tensor_copy(tb[:, :n], t_tile[:, :n])
                elif pool_cast == "act":
                    nc.scalar.activation(tb[:, :n], t_tile[:, :n], AF.Copy)
                else:
                    nc.vector.tensor_copy(tb[:, :n], t_tile[:, :n])
                ms = []
                for ci, c in enumerate((1, 2, 3)):
                    m = workp.tile([P, nmax], BF16, tag=f"m{ci}")
                    nc.vector.tensor_scalar(
                        m[:, :n], tb[:, :n], float(c), None, ALU.is_equal
                    )
                    ms.append(m)
                state[j] = (n, tb, ms, e4, den)

            def tail(j):
                n, tb, ms, e4, den = state.pop(j)

                # Pool classes first so Pool can run while DVE does recip.
                np_here = n_pool if j < n_chunks - tail_no_pool else 0
                porder = [2, 1, 0][:np_here]
                a_tiles = {}
                for ci in porder:
                    c = ci + 1
                    a = workp.tile([P, nmax], BF16, tag=f"apool{ci}")
                    nc.gpsimd.tensor_tensor(
                        a[:, :n], ms[ci][:, :n],
                        e4[:, c * n : (c + 1) * n], op=ALU.mult,
                    )
                    a_tiles[ci] = a

                r32 = workp.tile([P, nmax], F32, tag="r32")
                rbf = workp.tile([P, nmax], BF16, tag="rbf")
                if j == n_chunks - 1:
                    # Jump the DVE queue the moment den lands on the last chunk.
                    with tc.high_priority():
                        nc.vector.reciprocal_approx_fast(r32[:, :n], den[:, :n])
                        nc.vector.tensor_copy(rbf[:, :n], r32[:, :n])
                else:
                    nc.vector.reciprocal_approx_fast(r32[:, :n], den[:, :n])
                    nc.vector.tensor_copy(rbf[:, :n], r32[:, :n])

                for ci, c in enumerate((1, 2, 3)):
                    if ci in a_tiles:
                        a = a_tiles[ci]
                    else:
                        a = workp.tile([P, nmax], BF16, tag="a")
                        nc.vector.tensor_tensor(
                            a[:, :n], ms[ci][:, :n],
                            e4[:, c * n : (c + 1) * n], op=ALU.mult,
                        )
                    pc = workp.tile([P, nmax], BF16, tag="pc")
                    z_eng = nc.gpsimd if (j < n_chunks - tail_no_pool and ci < n_pool_z) else nc.vector
                    z_eng.tensor_tensor(
                        pc[:, :n], a[:, :n], rbf[:, :n], op=ALU.mult
                    )
                    # S1 and S2 partials on PE: [128,1] += pc_s^T @ ones and
                    # [128,128] += pc_s^T @ pc_s (diagonal = sum of squares)
                    for s in range(n // P):
                        sl_pc = pc[:, s * P : (s + 1) * P]
                        st = red_done[ci] == 0
                        sp = red_done[ci] == n_red_total - 1
                        nc.tensor.matmul(
                            s1_p[ci][:], sl_pc, ones,
                            start=st, stop=sp, skip_group_check=True,
                        )
                        nc.tensor.matmul(
                            s2_p[ci][:], sl_pc, sl_pc,
                            start=st, stop=sp, skip_group_check=True,
                        )
                        red_done[ci] += 1
                    if red_done[ci] == n_red_total:
                        assemble(ci)

            head(0)
            for j in range(n_chunks):
                if j + 1 < n_chunks:
                    head(j + 1)
                tail(j)

            nc.sync.dma_start(s1_d.ap(), s1sb[:])

    nc.compile()
    return nc, n_chunks


_CACHED = {}


def _get_program():
    if "nc" not in _CACHED:
        _CACHED["nc"] = build_program()[0]
    return _CACHED["nc"], None


def finish_host(results, cnt):
    """results: per-core dicts with outs1 [P,8], class-major: col 2c = S1_c,
    col 2c+1 = S2_c for c in 0..2 (classes 1..3)."""
    s1 = np.zeros(3, dtype=np.float64)
    s2 = np.zeros(3, dtype=np.float64)
    for r in results:
        o = r["outs1"].astype(np.float64)
        s1 += o[:, 0:6:2].sum(axis=0)
        s2 += o[:, 1:6:2].sum(axis=0)
    mean = s1 / (cnt + EPS)
    var = (s2 - 2.0 * mean * s1 + cnt * mean * mean) / (cnt + EPS)
    intra = np.where(cnt > 0, var, 0.0).sum()
    return np.float32(intra / (C - 1))


def kernel(inputs: np.ndarray, targets: np.ndarray) -> np.ndarray:
    import ml_dtypes

    inputs = np.asarray(inputs, dtype=np.float32)
    targets = np.asarray(targets, dtype=np.int32)
    nc, _ = _get_program()
    consts = np.concatenate(
        [np.eye(P, dtype=np.float32), np.ones((P, 1), dtype=np.float32)], axis=1
    ).astype(ml_dtypes.bfloat16)
    in_maps = [
        {
            "inputs": np.ascontiguousarray(inputs[i * B_LOC : (i + 1) * B_LOC]),
            "targets": np.ascontiguousarray(targets[i * B_LOC : (i + 1) * B_LOC]),
            "consts": consts,
        }
        for i in range(N_CORES)
    ]
    res = run_bass_kernel_spmd(nc, in_maps, list(range(N_CORES)))
    stats = [res.results[i] for i in range(N_CORES)]
    cnt = np.bincount(targets.ravel(), minlength=C)[1:C].astype(np.float64)
    return finish_host(stats, cnt)


# revision 57
# speedup vs baseline: 1.0085x; 1.0018x over previous
"""Trainium2 Bass kernel for nn_IntraClassLoss (segment_reduce).

Math: inputs [B,C,H,W] logits, targets [B,H,W] int labels, C=4.
probs = softmax(inputs, axis=1); for classes c in 1..C-1:
  cnt_c = #pixels with target==c
  S1_c  = sum over those pixels of p_c
  S2_c  = sum over those pixels of p_c^2
  mean_c = S1_c/(cnt_c+eps); var_c = (S2_c - 2*mean_c*S1_c + cnt_c*mean_c^2)/(cnt_c+eps)
  loss = sum_{c: cnt_c>0} var_c / (C-1)

Sharding: data-parallel over batch, 2 batches per core on 8 cores. Each core
reduces its shard to per-class S1/S2 partials which are DMA'd out and
finished on the host (cnt_c from a host-side bincount; no collectives).

Engine assignment (per [128,n] chunk; DMA of the 40MiB shard is the
roofline at ~117us, every engine is kept under it):
  DVE : tb=bf16(t) cast, masks m_c=(tb==c) (tensor_scalar 4x), recip(den),
        rbf=bf16(r), products a_c=m_c*e_c and pc_c=a_c*rbf (tensor_tensor 2x)
  ACT : exp only -- one fused op over all 4 class slices of x
  Pool: a_c for n_pool of the 3 classes (idle engine, off critical path)
  PE  : den = sum_c e_c (identity-matmul PSUM accumulation);
        S1_c via pc-stationary x ones matmuls into a [128,1] PSUM;
        S2_c via pc_slice^T @ pc_slice matmuls into a [128,128] PSUM whose
        accumulated DIAGONAL holds per-column sums of pc^2 (extracted once
        at the end with an identity mask + reduce)

The per-chunk work is emitted software-pipelined -- chunk j+1's DMA/exp/den
("head") is issued before chunk j's recip/product chain ("tail") -- so each
engine's in-order stream never makes next-chunk work wait on the previous
chunk's tail. On the final chunk, exp runs per-class interleaved with den
matmuls to shorten the post-last-DMA critical chain.
"""

import numpy as np

import concourse.bass as bass
import concourse.bacc as bacc
import concourse.tile as tile
from concourse import mybir
from concourse.bass_utils import run_bass_kernel_spmd

F32 = mybir.dt.float32
BF16 = mybir.dt.bfloat16
I32 = mybir.dt.int32
AF = mybir.ActivationFunctionType
ALU = mybir.AluOpType

B, C, H, W = 16, 4, 1024, 1024
N_CORES = 8
B_LOC = B // N_CORES
P = 128
EPS = 1e-6

CHUNKS = (512,) * 16  # per batch plane; must sum to H*W/P = 8192
N_POOL = 2  # how many of the 3 per-class a_c=m_c*e_c products run on Pool


def build_program(b_loc=B_LOC, h=H, w=W, chunks=CHUNKS, n_pool=N_POOL,
                  io_bufs=6, e_bufs=6, work_bufs=5, den_bufs=1,
                  pool_cast=False, tail_no_pool=0, n_pool_z=0, act_rbf=False):
    """Build the per-core SPMD program. Returns (nc, n_chunks_total)."""
    plane = h * w
    free = plane // P
    assert sum(chunks) == free
    all_chunks = [(b, n) for b in range(b_loc) for n in chunks]
    n_chunks = len(all_chunks)
    nmax = max(chunks)
    n_red_total = sum(n // P for _, n in all_chunks)  # PE-reduce matmuls/class

    nc = bacc.Bacc("TRN2", target_bir_lowering=False, debug=False)

    inputs_d = nc.dram_tensor("inputs", [b_loc, C, h, w], F32, kind="ExternalInput")
    targets_d = nc.dram_tensor("targets", [b_loc, h, w], I32, kind="ExternalInput")
    consts_d = nc.dram_tensor("consts", [P, P + 1], BF16, kind="ExternalInput")
    s1_d = nc.dram_tensor("outs1", [P, 8], F32, kind="ExternalOutput")

    with tile.TileContext(nc) as tc:
        with (
            tc.tile_pool(name="const", bufs=1) as constp,
            tc.tile_pool(name="io", bufs=io_bufs) as iop,
            tc.tile_pool(name="ework", bufs=e_bufs) as ep,
            tc.tile_pool(name="work", bufs=work_bufs) as workp,
            tc.tile_pool(name="stats", bufs=1) as statp,
            tc.tile_pool(name="psum", bufs=den_bufs, space="PSUM") as psump,
            tc.tile_pool(name="psum1", bufs=1, space="PSUM") as psump1,
        ):
            consts = constp.tile([P, P + 1], BF16)
            nc.sync.dma_start(consts[:], consts_d.ap())
            ident = consts[:, :P]
            ones = consts[:, P : P + 1]

            s1_p = [
                psump1.tile([P, 1], F32, tag=f"s1p{ci}", name=f"s1p{ci}")
                for ci in range(3)
            ]
            s2_p = [
                psump1.tile([P, P], F32, tag=f"s2p{ci}", name=f"s2p{ci}")
                for ci in range(3)
            ]
            red_done = [0, 0, 0]

            # Output staging, class-major: cols 2ci = S1_c, 2ci+1 = S2_c.
            s1sb = statp.tile([P, 8], F32, tag="s1sb")
            nc.vector.memset(s1sb[:], 0.0)

            def assemble(ci):
                diag = statp.tile([P, P], F32, tag=f"diag{ci}")
                nc.vector.tensor_copy(s1sb[:, 2 * ci : 2 * ci + 1], s1_p[ci][:])
                # diag extract + reduce in one op: (s2_p * 1) * ident, accum
                nc.vector.scalar_tensor_tensor(
                    out=diag[:], in0=s2_p[ci][:], scalar=1.0, in1=ident,
                    op0=ALU.mult, op1=ALU.mult,
                    accum_out=s1sb[:, 2 * ci + 1 : 2 * ci + 2],
                )


            state = {}

            def head(j):
                b, n = all_chunks[j]
                off = sum(nn for _, nn in all_chunks[:j]) % free
                sl = slice(off, off + n)

                t_tile = iop.tile([P, nmax], I32, tag="t")
                tgt_ap = targets_d.ap()[b].rearrange("(p a) w -> p (a w)", p=P)
                nc.sync.dma_start(t_tile[:, :n], tgt_ap[:, sl])

                x4 = iop.tile([P, 4 * nmax], F32, tag="x4")
                split_here = j >= n_chunks - 2
                corder = list(range(C))
                for c in corder:
                    x_ap = inputs_d.ap()[b, c].rearrange("(p a) w -> p (a w)", p=P)
                    nc.sync.dma_start(x4[:, c * n : (c + 1) * n], x_ap[:, sl])
                e4 = ep.tile([P, 4 * nmax], BF16, tag="e4")
                den = psump.tile([P, nmax], F32, tag="den")
                nsl = (n + 511) // 512
                if not split_here:
                    nc.scalar.activation(e4[:, : 4 * n], x4[:, : 4 * n], AF.Exp)
                for k, c in enumerate(corder):
                    if split_here:
                        nc.scalar.activation(
                            e4[:, c * n : (c + 1) * n],
                            x4[:, c * n : (c + 1) * n],
                            AF.Exp,
                        )
                    for hh in range(nsl):
                        s2 = slice(hh * 512, min((hh + 1) * 512, n))
                        nc.tensor.matmul(
                            den[:, s2],
                            ident,
                            e4[:, c * n + s2.start : c * n + s2.stop],
                            start=(k == 0),
                            stop=(k == C - 1),
                            skip_group_check=True,
                        )
                state[j] = (n, t_tile, e4, den)

            def tail(j):
                n, t_tile, e4, den = state.pop(j)

                # recip/rbf first: they gate this chunk's whole product chain.
                r32 = workp.tile([P, nmax], F32, tag="r32")
                nc.vector.reciprocal_approx_fast(r32[:, :n], den[:, :n])
                rbf = workp.tile([P, nmax], BF16, tag="rbf")
                if act_rbf:
                    nc.scalar.activation(rbf[:, :n], r32[:, :n], AF.Copy)
                else:
                    nc.vector.tensor_copy(rbf[:, :n], r32[:, :n])

                tb = workp.tile([P, nmax], BF16, tag="tb")
                if pool_cast == "pool":
                    nc.gpsimd.tensor_copy(tb[:, :n], t_tile[:, :n])
                elif pool_cast == "act":
                    nc.scalar.activation(tb[:, :n], t_tile[:, :n], AF.Copy)
                else:
                    nc.vector.tensor_copy(tb[:, :n], t_tile[:, :n])
                ms = []
                for ci, c in enumerate((1, 2, 3)):
                    m = workp.tile([P, nmax], BF16, tag=f"m{ci}")
                    nc.vector.tensor_scalar(
                        m[:, :n], tb[:, :n], float(c), None, ALU.is_equal
                    )
                    ms.append(m)

                # Pool classes first so Pool can run while DVE does recip.
                np_here = n_pool if j < n_chunks - tail_no_pool else 0
                porder = [2, 1, 0][:np_here]
                a_tiles = {}
                for ci in porder:
                    c = ci + 1
                    a = workp.tile([P, nmax], BF16, tag=f"apool{ci}")
                    nc.gpsimd.tensor_tensor(
                        a[:, :n], ms[ci][:, :n],
                        e4[:, c * n : (c + 1) * n], op=ALU.mult,
                    )
                    a_tiles[ci] = a

                for ci, c in enumerate((1, 2, 3)):
                    if ci in a_tiles:
                        a = a_tiles[ci]
                    else:
                        a = workp.tile([P, nmax], BF16, tag="a")
                        nc.vector.tensor_tensor(
                            a[:, :n], ms[ci][:, :n],
                            e4[:, c * n : (c + 1) * n], op=ALU.mult,
                        )
                    pc = workp.tile([P, nmax], BF16, tag="pc")
                    z_eng = nc.gpsimd if ci < n_pool_z else nc.vector
                    z_eng.tensor_tensor(
                        pc[:, :n], a[:, :n], rbf[:, :n], op=ALU.mult
                    )
                    # S1 and S2 partials on PE: [128,1] += pc_s^T @ ones and
                    # [128,128] += pc_s^T @ pc_s (diagonal = sum of squares)
                    for s in range(n // P):
                        sl_pc = pc[:, s * P : (s + 1) * P]
                        st = red_done[ci] == 0
                        sp = red_done[ci] == n_red_total - 1
                        nc.tensor.matmul(
                            s1_p[ci][:], sl_pc, ones,
                            start=st, stop=sp, skip_group_check=True,
                        )
                        nc.tensor.matmul(
                            s2_p[ci][:], sl_pc, sl_pc,
                            start=st, stop=sp, skip_group_check=True,
                        )
                        red_done[ci] += 1
                    if red_done[ci] == n_red_total:
                        assemble(ci)

            head(0)
            for j in range(n_chunks):
                if j + 1 < n_chunks:
                    head(j + 1)
                tail(j)

            nc.sync.dma_start(s1_d.ap(), s1sb[:])

    nc.compile()
    return nc, n_chunks


_CACHED = {}


def _get_program():
    if "nc" not in _CACHED:
        _CACHED["nc"] = build_program()[0]
    return _CACHED["nc"], None


def finish_host(results, cnt):
    """results: per-core dicts with outs1 [P,8], class-major: col 2c = S1_c,
    col 2c+1 = S2_c for c in 0..2 (classes 1..3)."""
    s1 = np.zeros(3, dtype=np.float64)
    s2 = np.zeros(3, dtype=np.float64)
    for r in results:
        o = r["outs1"].astype(np.float64)
        s1 += o[:, 0:6:2].sum(axis=0)
        s2 += o[:, 1:6:2].sum(axis=0)
    mean = s1 / (cnt + EPS)
    var = (s2 - 2.0 * mean * s1 + cnt * mean * mean) / (cnt + EPS)
    intra = np.where(cnt > 0, var, 0.0).sum()
    return np.float32(intra / (C - 1))


def kernel(inputs: np.ndarray, targets: np.ndarray) -> np.ndarray:
    import ml_dtypes

    inputs = np.asarray(inputs, dtype=np.float32)
    targets = np.asarray(targets, dtype=np.int32)
    nc, _ = _get_program()
    consts = np.concatenate(
        [np.eye(P, dtype=np.float32), np.ones((P, 1), dtype=np.float32)], axis=1
    ).astype(ml_dtypes.bfloat16)
    in_maps = [
        {
            "inputs": np.ascontiguousarray(inputs[i * B_LOC : (i + 1) * B_LOC]),
            "targets": np.ascontiguousarray(targets[i * B_LOC : (i + 1) * B_LOC]),
            "consts": consts,
        }
        for i in range(N_CORES)
    ]
    res = run_bass_kernel_spmd(nc, in_maps, list(range(N_CORES)))
    stats = [res.results[i] for i in range(N_CORES)]
    cnt = np.bincount(targets.ravel(), minlength=C)[1:C].astype(np.float64)
    return finish_host(stats, cnt)
